# revision 15
# baseline (speedup 1.0000x reference)
"""DynamicConv2d Trainium2 kernel.

Full inputs -> shard batch across 8 NeuronCores (data parallel) -> bass/Tile
kernel per core -> gather.

Per-core layout: channels on partitions (2 tiles of 128 for C=256).
  1. pooling: DVE block-sum reduces -> pooledT [128, b_loc*10] (9 pool cols +
     gap col per sample); 1/256 folded into w1, gap col = sum(pool cols)/9.
  2. weight synthesis on PE: h = (w1/256)^T-matmul, BN+GELU fused into one
     ScalarE activation (per-partition scale/bias), second matmul with
     [w2^T; b2] (bias as a 65th contraction row), Exp on ScalarE, softmax
     normalization + dyn_weight contraction on DVE -> Wb [128, b_loc*10]
     (9 conv-tap weights + 1 bias column per sample).
  3. depthwise 3x3 conv: 9 taps as diagonal-matrix f32r matmuls accumulating
     in PSUM (diag = identity * per-partition scalar, built on DVE); image
     edges handled by clipping the matmul APs (no padded copy, so HBM DMAs
     stay fully contiguous). PSUM evacuation + per-channel dynamic bias in a
     single ScalarE activation.
"""

import numpy as np

import concourse.bacc as bacc
import concourse.bass as bass
import concourse.mybir as mybir
import concourse.tile as tile
from concourse.bass_utils import run_bass_kernel_spmd

f32 = mybir.dt.float32
f32r = mybir.dt.float32r
AF = mybir.ActivationFunctionType
ALU = mybir.AluOpType
AX = mybir.AxisListType

N_CORES = 8
C, H, W = 256, 48, 48
HW = H * W
G = 4
CR = 64          # C // reduction_ratio
NTAP = 9
ROWS = 8         # output rows per matmul chunk (8*48=384 <= 512 psum bank)
EPS = 1e-5

TAPS = [(ky - 1, kx - 1) for ky in range(3) for kx in range(3)]
TAP_ORDER = [4] + [t for t in range(NTAP) if t != 4]  # center tap first: full
# coverage => its matmuls carry start=True and set has_written for the bank.
XPAD = 49        # zero pad elems each side of the flattened image: every tap
# reads a plain shifted stride-1 window; out-of-image reads hit zeros (top/
# bottom) or wrapped row-edge junk that gets subtracted after evacuation.
CHUNKS = [(0, 1024), (1024, 1024), (2048, 256)]  # psum tiles per unit


def _body(nc, b_loc, x_d, w1t_d, bns_d, bnb_d, w2tb_d, dynrep_d, ident_d, out_d, tc):
    npairs = b_loc // 2
    with (
        tc.tile_pool(name="constp", bufs=1) as constp,
        tc.tile_pool(name="xp", bufs=12) as xp,
        tc.tile_pool(name="outp", bufs=3) as outp,
        tc.tile_pool(name="diagp", bufs=12) as diagp,
        tc.tile_pool(name="smallp", bufs=12) as smallp,
        tc.tile_pool(name="convps", bufs=3, space="PSUM") as convps,
        tc.tile_pool(name="convps2", bufs=1, space="PSUM") as convps2,
        tc.tile_pool(name="synps", bufs=1, space="PSUM") as synps,
    ):
        consts = {}

        def load_consts():
            ident = constp.tile([128, 128], f32, name="ident")
            nc.sync.dma_start(out=ident[:], in_=ident_d[:])
            w1t = []
            for i in range(2):
                w1t_i = constp.tile([128, CR], f32, name=f"w1t{i}")
                nc.sync.dma_start(out=w1t_i[:],
                                  in_=w1t_d[i * 128:(i + 1) * 128, :])
                w1t.append(w1t_i)
            bns = constp.tile([CR, 1], f32, name="bns")
            nc.sync.dma_start(out=bns[:], in_=bns_d[:])
            bnb = constp.tile([CR, 1], f32, name="bnb")
            nc.sync.dma_start(out=bnb[:], in_=bnb_d[:])
            w2tb = constp.tile([CR + 1, G * C], f32, name="w2tb")
            nc.sync.dma_start(out=w2tb[:], in_=w2tb_d[:])
            dynrep = {}
            for g in range(G):
                for ch in range(2):
                    t = constp.tile([128, b_loc * 10], f32,
                                    name=f"dynrep{g}{ch}")
                    nc.sync.dma_start(out=t[:], in_=dynrep_d[g, ch])
                    dynrep[(g, ch)] = t
            pooledT = [constp.tile([128, b_loc * 10], f32,
                                   name=f"pooledT{ch}") for ch in range(2)]
            expsb = [constp.tile([128, b_loc * 10], f32, name=f"expsb{gi}")
                     for gi in range(2 * G)]
            Wb = [constp.tile([128, b_loc * 10], f32, name=f"Wb{ch}")
                  for ch in range(2)]
            geluh = constp.tile([CR + 1, b_loc * 10], f32, name="geluh")
            nc.vector.memset(geluh[CR:CR + 1, :], 1.0)  # ones row adds b2
            consts.update(ident=ident, w1t=w1t, bns=bns, bnb=bnb, w2tb=w2tb,
                          dynrep=dynrep, pooledT=pooledT, expsb=expsb, Wb=Wb,
                          geluh=geluh)

        def load_unit(b, ch):
            # f32r tile (the BIR verifier wants fp32r matmul inputs in fp32r
            # locations); DVE/GPS read it via f32 views. XPAD zero elems on
            # both sides let every tap read a plain shifted window.
            xt = xp.tile([128, HW + 2 * XPAD], f32r, tag="x",
                         name=f"x{b}c{ch}")
            nc.gpsimd.memset(xt.bitcast(f32)[:, 0:XPAD], 0.0)
            nc.gpsimd.memset(xt.bitcast(f32)[:, XPAD + HW:], 0.0)
            nc.sync.dma_start(
                out=xt[:, XPAD:XPAD + HW],
                in_=x_d[b, ch * 128:(ch + 1) * 128].rearrange(
                    "c h w -> c (h w)").bitcast(f32r))
            return xt

        def pool_unit(b, ch, xt):
            xv = xt.bitcast(f32)[:, XPAD:XPAD + HW].rearrange(
                "p (ky h kx w) -> p ky kx h w", ky=3, h=16, kx=3, w=16)
            pc = consts["pooledT"][ch]
            o = b * 10
            for ky in range(3):
                nc.vector.tensor_reduce(
                    out=pc[:, o + ky * 3: o + ky * 3 + 3],
                    in_=xv[:, ky], axis=AX.XY, op=ALU.add)
            gtmp = smallp.tile([128, 1], f32, tag="gt", name="gtmp")
            nc.vector.tensor_reduce(
                out=gtmp[:], in_=pc[:, o:o + 9], axis=AX.X, op=ALU.add)
            nc.gpsimd.tensor_scalar_mul(
                pc[:, o + 9:o + 10], gtmp[:], 1.0 / 9.0)

        x_t = {}
        for pr in range(npairs):
            bs = (2 * pr, 2 * pr + 1)
            c0 = bs[0] * 10

            # ---------------- load + pooling ----------------
            for b in bs:
                for ch in range(2):
                    x_t[(b, ch)] = load_unit(b, ch)
            if pr == 0:
                load_consts()
            ident = consts["ident"]
            w1t, bns, bnb = consts["w1t"], consts["bns"], consts["bnb"]
            w2tb, dynrep = consts["w2tb"], consts["dynrep"]
            pooledT, expsb, Wb = (consts["pooledT"], consts["expsb"],
                                  consts["Wb"])
            geluh = consts["geluh"]
            for b in bs:
                for ch in range(2):
                    pool_unit(b, ch, x_t[(b, ch)])

            # ---------------- synthesis (per pair, 20 cols) ----------------
            hpsum = synps.tile([128, 20], f32, tag="syn", name="hpsum")
            for ch in range(2):
                nc.tensor.matmul(
                    out=hpsum[0:CR, :],
                    lhsT=w1t[ch][:],
                    rhs=pooledT[ch][:, c0:c0 + 20],
                    start=(ch == 0), stop=(ch == 1))
            nc.scalar.activation(
                out=geluh[0:CR, c0:c0 + 20], in_=hpsum[0:CR, :],
                func=AF.Gelu, bias=bnb[:], scale=bns[:])
            for gi in range(2 * G):
                o2 = synps.tile([128, 20], f32, tag="syn", name="o2")
                nc.tensor.matmul(
                    out=o2[:],
                    lhsT=w2tb[:, gi * 128:(gi + 1) * 128],
                    rhs=geluh[:, c0:c0 + 20],
                    start=True, stop=True)
                nc.scalar.activation(
                    out=expsb[gi][:, c0:c0 + 20], in_=o2[:], func=AF.Exp)

            # softmax over G + dyn contraction -> Wb cols (DVE)
            for ch in range(2):
                es = [expsb[g * 2 + ch][:, c0:c0 + 20] for g in range(G)]
                ds = [dynrep[(g, ch)][:, c0:c0 + 20] for g in range(G)]
                s01 = smallp.tile([128, 20], f32, tag="sm", name="s01")
                s23 = smallp.tile([128, 20], f32, tag="sm", name="s23")
                ssum = smallp.tile([128, 20], f32, tag="sm", name="ssum")
                nc.vector.tensor_add(s01[:], es[0], es[1])
                nc.vector.tensor_add(s23[:], es[2], es[3])
                nc.vector.tensor_add(ssum[:], s01[:], s23[:])
                rec = smallp.tile([128, 20], f32, tag="sm", name="rec")
                nc.vector.reciprocal(rec[:], ssum[:])
                wa = smallp.tile([128, 20], f32, tag="sm", name="wa0")
                nc.vector.tensor_mul(wa[:], es[0], ds[0])
                for g in range(1, G):
                    tm = smallp.tile([128, 20], f32, tag="sm", name=f"tm{g}")
                    nc.vector.tensor_mul(tm[:], es[g], ds[g])
                    wa2 = smallp.tile([128, 20], f32, tag="sm", name=f"wa{g}")
                    nc.vector.tensor_add(wa2[:], wa[:], tm[:])
                    wa = wa2
                nc.vector.tensor_mul(Wb[ch][:, c0:c0 + 20], wa[:], rec[:])

            # ---------------- depthwise conv ----------------
            for b in bs:
                for ch in range(2):
                    xt = x_t.pop((b, ch))
                    diags = []
                    for t in range(NTAP):
                        dg = diagp.tile([128, 128], f32r, tag="dg",
                                        name=f"dg{t}")
                        nc.gpsimd.tensor_scalar_mul(
                            dg[:], ident[:],
                            Wb[ch][:, b * 10 + t:b * 10 + t + 1])
                        diags.append(dg)
                    negW = smallp.tile([128, NTAP], f32, tag="ng", name="negW")
                    nc.gpsimd.tensor_scalar_mul(
                        negW[:], Wb[ch][:, b * 10:b * 10 + NTAP], -1.0)
                    ot = outp.tile([128, HW], f32, tag="ot", name="ot")
                    pts = [convps.tile([128, 1024], f32, tag="cps",
                                       name=f"cpt{j}") for j in range(2)]
                    pts.append(convps2.tile([128, 256], f32, tag="cp2",
                                            name="cpt2"))
                    # every tap covers all HW outputs as flat stride-1
                    # matmuls over the padded x; junk in out cols 0 / W-1
                    # (row-edge wrap) is subtracted after evacuation.
                    for idx, ti in enumerate(TAP_ORDER):
                        dy, dx = TAPS[ti]
                        sh = XPAD + W * dy + dx
                        for (e0, clen), pt in zip(CHUNKS, pts):
                            for s0 in range(0, clen, 512):
                                n = min(512, clen - s0)
                                nc.tensor.matmul(
                                    out=pt[:, s0:s0 + n],
                                    lhsT=diags[ti][:],
                                    rhs=xt[:, sh + e0 + s0:sh + e0 + s0 + n],
                                    start=(idx == 0), stop=(idx == NTAP - 1))
                    for (e0, clen), pt in zip(CHUNKS, pts):
                        nc.scalar.activation(
                            out=ot[:, e0:e0 + clen],
                            in_=pt[:, 0:clen],
                            func=AF.Identity,
                            bias=Wb[ch][:, b * 10 + 9:b * 10 + 10],
                            scale=1.0)
                    # fix the two wrapped edge columns (reads x as f32)
                    ov = ot.rearrange("p (h w) -> p h w", w=W)
                    xf = xt.bitcast(f32)[:, XPAD:XPAD + HW].rearrange(
                        "p (h w) -> p h w", w=W)
                    for dy in (-1, 0, 1):
                        # col 0 got x(h+dy-1, W-1) * w[(dy,-1)] where the
                        # source pixel exists
                        ti = (dy + 1) * 3 + 0
                        hs, he = max(0, 1 - dy), H
                        nc.vector.scalar_tensor_tensor(
                            out=ov[:, hs:he, 0:1],
                            in0=xf[:, hs + dy - 1:he + dy - 1, W - 1:W],
                            scalar=negW[:, ti:ti + 1],
                            in1=ov[:, hs:he, 0:1],
                            op0=ALU.mult, op1=ALU.add)
                        # col W-1 got x(h+dy+1, 0) * w[(dy,+1)]
                        ti = (dy + 1) * 3 + 2
                        hs, he = 0, min(H, H - 1 - dy)
                        nc.vector.scalar_tensor_tensor(
                            out=ov[:, hs:he, W - 1:W],
                            in0=xf[:, hs + dy + 1:he + dy + 1, 0:1],
                            scalar=negW[:, ti:ti + 1],
                            in1=ov[:, hs:he, W - 1:W],
                            op0=ALU.mult, op1=ALU.add)
                    nc.scalar.dma_start(
                        out=out_d[b, ch * 128:(ch + 1) * 128].rearrange(
                            "c h w -> c (h w)"),
                        in_=ot[:])


def build_nc(b_loc):
    # Bacc (not raw Bass): its compile() pass splits multi-sem waits into
    # EventSemaphore instructions and moves matmul waits onto LDWEIGHTS —
    # without it walrus rejects Tile output ("Too many sync wait commands").
    nc = bacc.Bacc("TRN2", target_bir_lowering=False, debug=False)
    x_d = nc.dram_tensor("x", [b_loc, C, H, W], f32, kind="ExternalInput").ap()
    w1t_d = nc.dram_tensor("w1t", [C, CR], f32, kind="ExternalInput").ap()
    bns_d = nc.dram_tensor("bns", [CR, 1], f32, kind="ExternalInput").ap()
    bnb_d = nc.dram_tensor("bnb", [CR, 1], f32, kind="ExternalInput").ap()
    w2tb_d = nc.dram_tensor("w2tb", [CR + 1, G * C], f32,
                            kind="ExternalInput").ap()
    dynrep_d = nc.dram_tensor("dynrep", [G, 2, 128, b_loc * 10], f32,
                              kind="ExternalInput").ap()
    ident_d = nc.dram_tensor("ident", [128, 128], f32,
                             kind="ExternalInput").ap()
    out_d = nc.dram_tensor("out", [b_loc, C, H, W], f32,
                           kind="ExternalOutput").ap()
    with tile.TileContext(nc) as tc:
        _body(nc, b_loc, x_d, w1t_d, bns_d, bnb_d, w2tb_d, dynrep_d, ident_d,
              out_d, tc)
    nc.compile()
    return nc


_NC_CACHE = {}


def get_nc(b_loc):
    if b_loc not in _NC_CACHE:
        _NC_CACHE[b_loc] = build_nc(b_loc)
    return _NC_CACHE[b_loc]


def make_host_inputs(b_loc, w1, bn_gamma, bn_beta, bn_mean, bn_var, w2, b2,
                     dyn_weight, dyn_bias):
    w1 = np.asarray(w1, np.float32)
    inv = np.asarray(bn_gamma, np.float32) / np.sqrt(
        np.asarray(bn_var, np.float32) + EPS)
    bns = inv.reshape(CR, 1).astype(np.float32)
    bnb = (np.asarray(bn_beta, np.float32)
           - np.asarray(bn_mean, np.float32) * inv).reshape(CR, 1)
    bnb = bnb.astype(np.float32)
    w1t = np.ascontiguousarray(w1.T / 256.0).astype(np.float32)
    w2tb = np.concatenate(
        [np.asarray(w2, np.float32).T,
         np.asarray(b2, np.float32)[None, :]], axis=0)
    w2tb = np.ascontiguousarray(w2tb).astype(np.float32)
    dw = np.asarray(dyn_weight, np.float32).reshape(G, C, NTAP)
    db = np.asarray(dyn_bias, np.float32)
    dynrep = np.zeros((G, 2, 128, b_loc * 10), np.float32)
    for ch in range(2):
        cs = slice(ch * 128, (ch + 1) * 128)
        for b in range(b_loc):
            dynrep[:, ch, :, b * 10:b * 10 + 9] = dw[:, cs, :]
            dynrep[:, ch, :, b * 10 + 9] = db[:, cs]
    ident = np.eye(128, dtype=np.float32)
    return dict(w1t=w1t, bns=bns, bnb=bnb, w2tb=w2tb, dynrep=dynrep,
                ident=ident)


def kernel(x, w1, bn_gamma, bn_beta, bn_mean, bn_var, w2, b2, dyn_weight,
           dyn_bias):
    x = np.asarray(x, np.float32)
    B = x.shape[0]
    b_loc = B // N_CORES
    nc = get_nc(b_loc)
    params = make_host_inputs(b_loc, w1, bn_gamma, bn_beta, bn_mean, bn_var,
                              w2, b2, dyn_weight, dyn_bias)
    in_maps = []
    for cid in range(N_CORES):
        m = dict(params)
        m["x"] = np.ascontiguousarray(x[cid * b_loc:(cid + 1) * b_loc])
        in_maps.append(m)
    res = run_bass_kernel_spmd(nc, in_maps, list(range(N_CORES)))
    outs = [res.results[i]["out"] for i in range(N_CORES)]
    return np.concatenate(outs, axis=0).astype(np.float32)


# revision 16
# speedup vs baseline: 1.5971x; 1.5971x over previous
"""DynamicConv2d Trainium2 kernel.

Full inputs -> shard batch across 8 NeuronCores (data parallel) -> bass/Tile
kernel per core -> gather.

Per-core layout: channels on partitions (2 tiles of 128 for C=256).
  1. pooling: DVE block-sum reduces -> pooledT [128, b_loc*10] (9 pool cols +
     gap col per sample); 1/256 folded into w1, gap col = sum(pool cols)/9.
  2. weight synthesis on PE: h = (w1/256)^T-matmul, BN+GELU fused into one
     ScalarE activation (per-partition scale/bias), second matmul with
     [w2^T; b2] (bias as a 65th contraction row), Exp on ScalarE, softmax
     normalization + dyn_weight contraction on DVE -> Wb [128, b_loc*10]
     (9 conv-tap weights + 1 bias column per sample).
  3. depthwise 3x3 conv: 9 taps as diagonal-matrix f32r matmuls accumulating
     in PSUM (diag = identity * per-partition scalar, built on DVE); image
     edges handled by clipping the matmul APs (no padded copy, so HBM DMAs
     stay fully contiguous). PSUM evacuation + per-channel dynamic bias in a
     single ScalarE activation.
"""

import numpy as np

import concourse.bacc as bacc
import concourse.bass as bass
import concourse.mybir as mybir
import concourse.tile as tile
from concourse.bass_utils import run_bass_kernel_spmd

f32 = mybir.dt.float32
f32r = mybir.dt.float32r
AF = mybir.ActivationFunctionType
ALU = mybir.AluOpType
AX = mybir.AxisListType

N_CORES = 8
C, H, W = 256, 48, 48
HW = H * W
G = 4
CR = 64          # C // reduction_ratio
NTAP = 9
ROWS = 8         # output rows per matmul chunk (8*48=384 <= 512 psum bank)
EPS = 1e-5

TAPS = [(ky - 1, kx - 1) for ky in range(3) for kx in range(3)]
TAP_ORDER = [4] + [t for t in range(NTAP) if t != 4]  # center tap first: full
# coverage => its matmuls carry start=True and set has_written for the bank.
XPAD = 49        # zero pad elems each side of the flattened image: every tap
# reads a plain shifted stride-1 window; out-of-image reads hit zeros (top/
# bottom) or wrapped row-edge junk that gets subtracted after evacuation.
CHUNKS = [(0, 1024), (1024, 1024), (2048, 256)]  # psum tiles per unit


def _body(nc, b_loc, x_d, w1t_d, bns_d, bnb_d, w2tb_d, dynrep_d, ident_d, out_d, tc):
    npairs = b_loc // 2
    with (
        tc.tile_pool(name="constp", bufs=1) as constp,
        tc.tile_pool(name="xp", bufs=12) as xp,
        tc.tile_pool(name="outp", bufs=3) as outp,
        tc.tile_pool(name="diagp", bufs=12) as diagp,
        tc.tile_pool(name="smallp", bufs=12) as smallp,
        tc.tile_pool(name="convps", bufs=3, space="PSUM") as convps,
        tc.tile_pool(name="convps2", bufs=1, space="PSUM") as convps2,
        tc.tile_pool(name="synps", bufs=1, space="PSUM") as synps,
    ):
        consts = {}

        def load_consts():
            ident = constp.tile([128, 128], f32, name="ident")
            nc.sync.dma_start(out=ident[:], in_=ident_d[:])
            w1t = []
            for i in range(2):
                w1t_i = constp.tile([128, CR], f32, name=f"w1t{i}")
                nc.sync.dma_start(out=w1t_i[:],
                                  in_=w1t_d[i * 128:(i + 1) * 128, :])
                w1t.append(w1t_i)
            bns = constp.tile([CR, 1], f32, name="bns")
            nc.sync.dma_start(out=bns[:], in_=bns_d[:])
            bnb = constp.tile([CR, 1], f32, name="bnb")
            nc.sync.dma_start(out=bnb[:], in_=bnb_d[:])
            w2tb = constp.tile([CR + 1, G * C], f32, name="w2tb")
            nc.sync.dma_start(out=w2tb[:], in_=w2tb_d[:])
            dynrep = {}
            for g in range(G):
                for ch in range(2):
                    t = constp.tile([128, b_loc * 10], f32,
                                    name=f"dynrep{g}{ch}")
                    nc.sync.dma_start(out=t[:], in_=dynrep_d[g, ch])
                    dynrep[(g, ch)] = t
            pooledT = [constp.tile([128, b_loc * 10], f32,
                                   name=f"pooledT{ch}") for ch in range(2)]
            expsb = [constp.tile([128, b_loc * 10], f32, name=f"expsb{gi}")
                     for gi in range(2 * G)]
            Wb = [constp.tile([128, b_loc * 10], f32, name=f"Wb{ch}")
                  for ch in range(2)]
            geluh = constp.tile([CR + 1, b_loc * 10], f32, name="geluh")
            nc.vector.memset(geluh[CR:CR + 1, :], 1.0)  # ones row adds b2
            consts.update(ident=ident, w1t=w1t, bns=bns, bnb=bnb, w2tb=w2tb,
                          dynrep=dynrep, pooledT=pooledT, expsb=expsb, Wb=Wb,
                          geluh=geluh)

        def load_unit(b, ch):
            # f32r tile (the BIR verifier wants fp32r matmul inputs in fp32r
            # locations); DVE/GPS read it via f32 views. XPAD zero elems on
            # both sides let every tap read a plain shifted window.
            xt = xp.tile([128, HW + 2 * XPAD], f32r, tag="x",
                         name=f"x{b}c{ch}")
            nc.gpsimd.memset(xt.bitcast(f32)[:, 0:XPAD], 0.0)
            nc.gpsimd.memset(xt.bitcast(f32)[:, XPAD + HW:], 0.0)
            nc.sync.dma_start(
                out=xt[:, XPAD:XPAD + HW],
                in_=x_d[b, ch * 128:(ch + 1) * 128].rearrange(
                    "c h w -> c (h w)").bitcast(f32r))
            return xt

        def pool_unit(b, ch, xt):
            xv = xt.bitcast(f32)[:, XPAD:XPAD + HW].rearrange(
                "p (ky h kx w) -> p ky kx h w", ky=3, h=16, kx=3, w=16)
            pc = consts["pooledT"][ch]
            o = b * 10
            for ky in range(3):
                nc.vector.tensor_reduce(
                    out=pc[:, o + ky * 3: o + ky * 3 + 3],
                    in_=xv[:, ky], axis=AX.XY, op=ALU.add)
            gtmp = smallp.tile([128, 1], f32, tag="gt", name="gtmp")
            nc.vector.tensor_reduce(
                out=gtmp[:], in_=pc[:, o:o + 9], axis=AX.X, op=ALU.add)
            nc.vector.tensor_scalar_mul(
                pc[:, o + 9:o + 10], gtmp[:], 1.0 / 9.0)

        x_t = {}
        for pr in range(npairs):
            bs = (2 * pr, 2 * pr + 1)
            c0 = bs[0] * 10

            # ---------------- load + pooling ----------------
            for b in bs:
                for ch in range(2):
                    x_t[(b, ch)] = load_unit(b, ch)
            if pr == 0:
                load_consts()
            ident = consts["ident"]
            w1t, bns, bnb = consts["w1t"], consts["bns"], consts["bnb"]
            w2tb, dynrep = consts["w2tb"], consts["dynrep"]
            pooledT, expsb, Wb = (consts["pooledT"], consts["expsb"],
                                  consts["Wb"])
            geluh = consts["geluh"]
            for b in bs:
                for ch in range(2):
                    pool_unit(b, ch, x_t[(b, ch)])

            # ---------------- synthesis (per pair, 20 cols) ----------------
            hpsum = synps.tile([128, 20], f32, tag="syn", name="hpsum")
            for ch in range(2):
                nc.tensor.matmul(
                    out=hpsum[0:CR, :],
                    lhsT=w1t[ch][:],
                    rhs=pooledT[ch][:, c0:c0 + 20],
                    start=(ch == 0), stop=(ch == 1))
            nc.scalar.activation(
                out=geluh[0:CR, c0:c0 + 20], in_=hpsum[0:CR, :],
                func=AF.Gelu, bias=bnb[:], scale=bns[:])
            for gi in range(2 * G):
                o2 = synps.tile([128, 20], f32, tag="syn", name="o2")
                nc.tensor.matmul(
                    out=o2[:],
                    lhsT=w2tb[:, gi * 128:(gi + 1) * 128],
                    rhs=geluh[:, c0:c0 + 20],
                    start=True, stop=True)
                nc.scalar.activation(
                    out=expsb[gi][:, c0:c0 + 20], in_=o2[:], func=AF.Exp)

            # softmax over G + dyn contraction -> Wb cols (DVE)
            for ch in range(2):
                es = [expsb[g * 2 + ch][:, c0:c0 + 20] for g in range(G)]
                ds = [dynrep[(g, ch)][:, c0:c0 + 20] for g in range(G)]
                s01 = smallp.tile([128, 20], f32, tag="sm", name="s01")
                s23 = smallp.tile([128, 20], f32, tag="sm", name="s23")
                ssum = smallp.tile([128, 20], f32, tag="sm", name="ssum")
                nc.vector.tensor_add(s01[:], es[0], es[1])
                nc.vector.tensor_add(s23[:], es[2], es[3])
                nc.vector.tensor_add(ssum[:], s01[:], s23[:])
                rec = smallp.tile([128, 20], f32, tag="sm", name="rec")
                nc.vector.reciprocal(rec[:], ssum[:])
                wa = smallp.tile([128, 20], f32, tag="sm", name="wa0")
                nc.vector.tensor_mul(wa[:], es[0], ds[0])
                for g in range(1, G):
                    tm = smallp.tile([128, 20], f32, tag="sm", name=f"tm{g}")
                    nc.vector.tensor_mul(tm[:], es[g], ds[g])
                    wa2 = smallp.tile([128, 20], f32, tag="sm", name=f"wa{g}")
                    nc.vector.tensor_add(wa2[:], wa[:], tm[:])
                    wa = wa2
                nc.vector.tensor_mul(Wb[ch][:, c0:c0 + 20], wa[:], rec[:])

            # ---------------- depthwise conv ----------------
            for b in bs:
                for ch in range(2):
                    xt = x_t.pop((b, ch))
                    diags = []
                    for t in range(NTAP):
                        dg = diagp.tile([128, 128], f32r, tag="dg",
                                        name=f"dg{t}")
                        nc.vector.tensor_scalar_mul(
                            dg[:], ident[:],
                            Wb[ch][:, b * 10 + t:b * 10 + t + 1])
                        diags.append(dg)
                    negW = smallp.tile([128, NTAP], f32, tag="ng", name="negW")
                    nc.vector.tensor_scalar_mul(
                        negW[:], Wb[ch][:, b * 10:b * 10 + NTAP], -1.0)
                    ot = outp.tile([128, HW], f32, tag="ot", name="ot")
                    pts = [convps.tile([128, 1024], f32, tag="cps",
                                       name=f"cpt{j}") for j in range(2)]
                    pts.append(convps2.tile([128, 256], f32, tag="cp2",
                                            name="cpt2"))
                    # every tap covers all HW outputs as flat stride-1
                    # matmuls over the padded x; junk in out cols 0 / W-1
                    # (row-edge wrap) is subtracted after evacuation.
                    for idx, ti in enumerate(TAP_ORDER):
                        dy, dx = TAPS[ti]
                        sh = XPAD + W * dy + dx
                        for (e0, clen), pt in zip(CHUNKS, pts):
                            for s0 in range(0, clen, 512):
                                n = min(512, clen - s0)
                                nc.tensor.matmul(
                                    out=pt[:, s0:s0 + n],
                                    lhsT=diags[ti][:],
                                    rhs=xt[:, sh + e0 + s0:sh + e0 + s0 + n],
                                    start=(idx == 0), stop=(idx == NTAP - 1))
                    for (e0, clen), pt in zip(CHUNKS, pts):
                        nc.scalar.activation(
                            out=ot[:, e0:e0 + clen],
                            in_=pt[:, 0:clen],
                            func=AF.Identity,
                            bias=Wb[ch][:, b * 10 + 9:b * 10 + 10],
                            scale=1.0)
                    # fix the two wrapped edge columns (reads x as f32)
                    ov = ot.rearrange("p (h w) -> p h w", w=W)
                    xf = xt.bitcast(f32)[:, XPAD:XPAD + HW].rearrange(
                        "p (h w) -> p h w", w=W)
                    for dy in (-1, 0, 1):
                        # col 0 got x(h+dy-1, W-1) * w[(dy,-1)] where the
                        # source pixel exists
                        ti = (dy + 1) * 3 + 0
                        hs, he = max(0, 1 - dy), H
                        nc.vector.scalar_tensor_tensor(
                            out=ov[:, hs:he, 0:1],
                            in0=xf[:, hs + dy - 1:he + dy - 1, W - 1:W],
                            scalar=negW[:, ti:ti + 1],
                            in1=ov[:, hs:he, 0:1],
                            op0=ALU.mult, op1=ALU.add)
                        # col W-1 got x(h+dy+1, 0) * w[(dy,+1)]
                        ti = (dy + 1) * 3 + 2
                        hs, he = 0, min(H, H - 1 - dy)
                        nc.vector.scalar_tensor_tensor(
                            out=ov[:, hs:he, W - 1:W],
                            in0=xf[:, hs + dy + 1:he + dy + 1, 0:1],
                            scalar=negW[:, ti:ti + 1],
                            in1=ov[:, hs:he, W - 1:W],
                            op0=ALU.mult, op1=ALU.add)
                    nc.scalar.dma_start(
                        out=out_d[b, ch * 128:(ch + 1) * 128].rearrange(
                            "c h w -> c (h w)"),
                        in_=ot[:])


def build_nc(b_loc):
    # Bacc (not raw Bass): its compile() pass splits multi-sem waits into
    # EventSemaphore instructions and moves matmul waits onto LDWEIGHTS —
    # without it walrus rejects Tile output ("Too many sync wait commands").
    nc = bacc.Bacc("TRN2", target_bir_lowering=False, debug=False)
    x_d = nc.dram_tensor("x", [b_loc, C, H, W], f32, kind="ExternalInput").ap()
    w1t_d = nc.dram_tensor("w1t", [C, CR], f32, kind="ExternalInput").ap()
    bns_d = nc.dram_tensor("bns", [CR, 1], f32, kind="ExternalInput").ap()
    bnb_d = nc.dram_tensor("bnb", [CR, 1], f32, kind="ExternalInput").ap()
    w2tb_d = nc.dram_tensor("w2tb", [CR + 1, G * C], f32,
                            kind="ExternalInput").ap()
    dynrep_d = nc.dram_tensor("dynrep", [G, 2, 128, b_loc * 10], f32,
                              kind="ExternalInput").ap()
    ident_d = nc.dram_tensor("ident", [128, 128], f32,
                             kind="ExternalInput").ap()
    out_d = nc.dram_tensor("out", [b_loc, C, H, W], f32,
                           kind="ExternalOutput").ap()
    with tile.TileContext(nc) as tc:
        _body(nc, b_loc, x_d, w1t_d, bns_d, bnb_d, w2tb_d, dynrep_d, ident_d,
              out_d, tc)
    nc.compile()
    return nc


_NC_CACHE = {}


def get_nc(b_loc):
    if b_loc not in _NC_CACHE:
        _NC_CACHE[b_loc] = build_nc(b_loc)
    return _NC_CACHE[b_loc]


def make_host_inputs(b_loc, w1, bn_gamma, bn_beta, bn_mean, bn_var, w2, b2,
                     dyn_weight, dyn_bias):
    w1 = np.asarray(w1, np.float32)
    inv = np.asarray(bn_gamma, np.float32) / np.sqrt(
        np.asarray(bn_var, np.float32) + EPS)
    bns = inv.reshape(CR, 1).astype(np.float32)
    bnb = (np.asarray(bn_beta, np.float32)
           - np.asarray(bn_mean, np.float32) * inv).reshape(CR, 1)
    bnb = bnb.astype(np.float32)
    w1t = np.ascontiguousarray(w1.T / 256.0).astype(np.float32)
    w2tb = np.concatenate(
        [np.asarray(w2, np.float32).T,
         np.asarray(b2, np.float32)[None, :]], axis=0)
    w2tb = np.ascontiguousarray(w2tb).astype(np.float32)
    dw = np.asarray(dyn_weight, np.float32).reshape(G, C, NTAP)
    db = np.asarray(dyn_bias, np.float32)
    dynrep = np.zeros((G, 2, 128, b_loc * 10), np.float32)
    for ch in range(2):
        cs = slice(ch * 128, (ch + 1) * 128)
        for b in range(b_loc):
            dynrep[:, ch, :, b * 10:b * 10 + 9] = dw[:, cs, :]
            dynrep[:, ch, :, b * 10 + 9] = db[:, cs]
    ident = np.eye(128, dtype=np.float32)
    return dict(w1t=w1t, bns=bns, bnb=bnb, w2tb=w2tb, dynrep=dynrep,
                ident=ident)


def kernel(x, w1, bn_gamma, bn_beta, bn_mean, bn_var, w2, b2, dyn_weight,
           dyn_bias):
    x = np.asarray(x, np.float32)
    B = x.shape[0]
    b_loc = B // N_CORES
    nc = get_nc(b_loc)
    params = make_host_inputs(b_loc, w1, bn_gamma, bn_beta, bn_mean, bn_var,
                              w2, b2, dyn_weight, dyn_bias)
    in_maps = []
    for cid in range(N_CORES):
        m = dict(params)
        m["x"] = np.ascontiguousarray(x[cid * b_loc:(cid + 1) * b_loc])
        in_maps.append(m)
    res = run_bass_kernel_spmd(nc, in_maps, list(range(N_CORES)))
    outs = [res.results[i]["out"] for i in range(N_CORES)]
    return np.concatenate(outs, axis=0).astype(np.float32)


# revision 18
# speedup vs baseline: 1.8174x; 1.1380x over previous
"""DynamicConv2d Trainium2 kernel.

Full inputs -> shard batch across 8 NeuronCores (data parallel) -> bass/Tile
kernel per core -> gather.

Per-core layout: channels on partitions (2 tiles of 128 for C=256).
  1. pooling: DVE block-sum reduces -> pooledT [128, b_loc*10] (9 pool cols +
     gap col per sample); 1/256 folded into w1, gap col = sum(pool cols)/9.
  2. weight synthesis on PE: h = (w1/256)^T-matmul, BN+GELU fused into one
     ScalarE activation (per-partition scale/bias), second matmul with
     [w2^T; b2] (bias as a 65th contraction row), Exp on ScalarE, softmax
     normalization + dyn_weight contraction on DVE -> Wb [128, b_loc*10]
     (9 conv-tap weights + 1 bias column per sample).
  3. depthwise 3x3 conv: 9 taps as diagonal-matrix f32r matmuls accumulating
     in PSUM (diag = identity * per-partition scalar, built on DVE); image
     edges handled by clipping the matmul APs (no padded copy, so HBM DMAs
     stay fully contiguous). PSUM evacuation + per-channel dynamic bias in a
     single ScalarE activation.
"""

import numpy as np

import concourse.bacc as bacc
import concourse.bass as bass
import concourse.mybir as mybir
import concourse.tile as tile
from concourse.bass_utils import run_bass_kernel_spmd

f32 = mybir.dt.float32
f32r = mybir.dt.float32r
AF = mybir.ActivationFunctionType
ALU = mybir.AluOpType
AX = mybir.AxisListType

N_CORES = 8
C, H, W = 256, 48, 48
HW = H * W
G = 4
CR = 64          # C // reduction_ratio
NTAP = 9
ROWS = 8         # output rows per matmul chunk (8*48=384 <= 512 psum bank)
EPS = 1e-5

TAPS = [(ky - 1, kx - 1) for ky in range(3) for kx in range(3)]
TAP_ORDER = [4] + [t for t in range(NTAP) if t != 4]  # center tap first: full
# coverage => its matmuls carry start=True and set has_written for the bank.
XPAD = 49        # zero pad elems each side of the flattened image: every tap
# reads a plain shifted stride-1 window; out-of-image reads hit zeros (top/
# bottom) or wrapped row-edge junk that gets subtracted after evacuation.
CHUNKS = [(0, 1024), (1024, 1024), (2048, 256)]  # psum tiles per unit


def _body(nc, b_loc, x_d, w1t_d, bns_d, bnb_d, w2tb_d, dynrep_d, ident_d, out_d, tc):
    npairs = b_loc // 2
    with (
        tc.tile_pool(name="constp", bufs=1) as constp,
        tc.tile_pool(name="xp", bufs=10) as xp,
        tc.tile_pool(name="outp", bufs=3) as outp,
        tc.tile_pool(name="diagp", bufs=72) as diagp,
        tc.tile_pool(name="smallp", bufs=12) as smallp,
        tc.tile_pool(name="convps", bufs=3, space="PSUM") as convps,
        tc.tile_pool(name="convps2", bufs=1, space="PSUM") as convps2,
        tc.tile_pool(name="synps", bufs=1, space="PSUM") as synps,
    ):
        consts = {}

        def load_consts():
            ident = constp.tile([128, 128], f32, name="ident")
            nc.sync.dma_start(out=ident[:], in_=ident_d[:])
            w1t = []
            for i in range(2):
                w1t_i = constp.tile([128, CR], f32, name=f"w1t{i}")
                nc.sync.dma_start(out=w1t_i[:],
                                  in_=w1t_d[i * 128:(i + 1) * 128, :])
                w1t.append(w1t_i)
            bns = constp.tile([CR, 1], f32, name="bns")
            nc.sync.dma_start(out=bns[:], in_=bns_d[:])
            bnb = constp.tile([CR, 1], f32, name="bnb")
            nc.sync.dma_start(out=bnb[:], in_=bnb_d[:])
            w2tb = constp.tile([CR + 1, G * C], f32, name="w2tb")
            nc.sync.dma_start(out=w2tb[:], in_=w2tb_d[:])
            dynrep = {}
            for g in range(G):
                for ch in range(2):
                    t = constp.tile([128, b_loc * 10], f32,
                                    name=f"dynrep{g}{ch}")
                    nc.sync.dma_start(out=t[:], in_=dynrep_d[g, ch])
                    dynrep[(g, ch)] = t
            pooledT = [constp.tile([128, b_loc * 10], f32,
                                   name=f"pooledT{ch}") for ch in range(2)]
            expsb = [constp.tile([128, b_loc * 10], f32, name=f"expsb{gi}")
                     for gi in range(2 * G)]
            Wb = [constp.tile([128, b_loc * 10], f32, name=f"Wb{ch}")
                  for ch in range(2)]
            geluh = constp.tile([CR + 1, b_loc * 10], f32, name="geluh")
            nc.vector.memset(geluh[CR:CR + 1, :], 1.0)  # ones row adds b2
            consts.update(ident=ident, w1t=w1t, bns=bns, bnb=bnb, w2tb=w2tb,
                          dynrep=dynrep, pooledT=pooledT, expsb=expsb, Wb=Wb,
                          geluh=geluh)

        def load_unit(b, ch):
            # f32r tile (the BIR verifier wants fp32r matmul inputs in fp32r
            # locations); DVE/GPS read it via f32 views. XPAD zero elems on
            # both sides let every tap read a plain shifted window.
            xt = xp.tile([128, HW + 2 * XPAD], f32r, tag="x",
                         name=f"x{b}c{ch}")
            nc.gpsimd.memset(xt.bitcast(f32)[:, 0:XPAD], 0.0)
            nc.gpsimd.memset(xt.bitcast(f32)[:, XPAD + HW:], 0.0)
            nc.sync.dma_start(
                out=xt[:, XPAD:XPAD + HW],
                in_=x_d[b, ch * 128:(ch + 1) * 128].rearrange(
                    "c h w -> c (h w)").bitcast(f32r))
            return xt

        def pool_unit(b, ch, xt):
            xv = xt.bitcast(f32)[:, XPAD:XPAD + HW].rearrange(
                "p (ky h kx w) -> p ky kx h w", ky=3, h=16, kx=3, w=16)
            pc = consts["pooledT"][ch]
            o = b * 10
            for ky in range(3):
                nc.vector.tensor_reduce(
                    out=pc[:, o + ky * 3: o + ky * 3 + 3],
                    in_=xv[:, ky], axis=AX.XY, op=ALU.add)
            gtmp = smallp.tile([128, 1], f32, tag="gt", name="gtmp")
            nc.vector.tensor_reduce(
                out=gtmp[:], in_=pc[:, o:o + 9], axis=AX.X, op=ALU.add)
            nc.vector.tensor_scalar_mul(
                pc[:, o + 9:o + 10], gtmp[:], 1.0 / 9.0)

        x_t = {}
        diag_t = {}
        negW_t = {}

        def synth_pair(pr):
            """Pooling-dependent weight synthesis + diag builds for pair pr.

            Emitted one pair ahead of that pair's conv so the PE never waits
            on the (in-order) DVE queue at pair boundaries.
            """
            bs = (2 * pr, 2 * pr + 1)
            c0 = bs[0] * 10
            ident = consts["ident"]
            w1t, bns, bnb = consts["w1t"], consts["bns"], consts["bnb"]
            w2tb, dynrep = consts["w2tb"], consts["dynrep"]
            pooledT, expsb, Wb = (consts["pooledT"], consts["expsb"],
                                  consts["Wb"])
            geluh = consts["geluh"]
            hpsum = synps.tile([128, 20], f32, tag="syn", name="hpsum")
            for ch in range(2):
                nc.tensor.matmul(
                    out=hpsum[0:CR, :],
                    lhsT=w1t[ch][:],
                    rhs=pooledT[ch][:, c0:c0 + 20],
                    start=(ch == 0), stop=(ch == 1))
            nc.scalar.activation(
                out=geluh[0:CR, c0:c0 + 20], in_=hpsum[0:CR, :],
                func=AF.Gelu, bias=bnb[:], scale=bns[:])
            for gi in range(2 * G):
                o2 = synps.tile([128, 20], f32, tag="syn", name="o2")
                nc.tensor.matmul(
                    out=o2[:],
                    lhsT=w2tb[:, gi * 128:(gi + 1) * 128],
                    rhs=geluh[:, c0:c0 + 20],
                    start=True, stop=True)
                nc.scalar.activation(
                    out=expsb[gi][:, c0:c0 + 20], in_=o2[:], func=AF.Exp)

            # softmax over G + dyn contraction -> Wb cols (DVE)
            for ch in range(2):
                es = [expsb[g * 2 + ch][:, c0:c0 + 20] for g in range(G)]
                ds = [dynrep[(g, ch)][:, c0:c0 + 20] for g in range(G)]
                s01 = smallp.tile([128, 20], f32, tag="sm", name="s01")
                s23 = smallp.tile([128, 20], f32, tag="sm", name="s23")
                ssum = smallp.tile([128, 20], f32, tag="sm", name="ssum")
                nc.vector.tensor_add(s01[:], es[0], es[1])
                nc.vector.tensor_add(s23[:], es[2], es[3])
                nc.vector.tensor_add(ssum[:], s01[:], s23[:])
                rec = smallp.tile([128, 20], f32, tag="sm", name="rec")
                nc.vector.reciprocal(rec[:], ssum[:])
                wa = smallp.tile([128, 20], f32, tag="sm", name="wa0")
                nc.vector.tensor_mul(wa[:], es[0], ds[0])
                for g in range(1, G):
                    tm = smallp.tile([128, 20], f32, tag="sm", name=f"tm{g}")
                    nc.vector.tensor_mul(tm[:], es[g], ds[g])
                    wa2 = smallp.tile([128, 20], f32, tag="sm", name=f"wa{g}")
                    nc.vector.tensor_add(wa2[:], wa[:], tm[:])
                    wa = wa2
                nc.vector.tensor_mul(Wb[ch][:, c0:c0 + 20], wa[:], rec[:])

            # diag + negW builds for all 4 units of the pair
            for b in bs:
                for ch in range(2):
                    diags = []
                    for t in range(NTAP):
                        dg = diagp.tile([128, 128], f32r, tag="dg",
                                        name=f"dg{t}")
                        nc.vector.tensor_scalar_mul(
                            dg[:], consts["ident"][:],
                            Wb[ch][:, b * 10 + t:b * 10 + t + 1])
                        diags.append(dg)
                    diag_t[(b, ch)] = diags
                    negW = smallp.tile([128, NTAP], f32, tag="ng",
                                       name="negW")
                    nc.vector.tensor_scalar_mul(
                        negW[:], Wb[ch][:, b * 10:b * 10 + NTAP], -1.0)
                    negW_t[(b, ch)] = negW

        for pr in range(npairs):
            if pr == 0:
                for b in (0, 1):
                    for ch in range(2):
                        x_t[(b, ch)] = load_unit(b, ch)
                load_consts()
                for b in (0, 1):
                    for ch in range(2):
                        pool_unit(b, ch, x_t[(b, ch)])
                synth_pair(0)
            if pr + 1 < npairs:
                for b in (2 * pr + 2, 2 * pr + 3):
                    for ch in range(2):
                        x_t[(b, ch)] = load_unit(b, ch)
                for b in (2 * pr + 2, 2 * pr + 3):
                    for ch in range(2):
                        pool_unit(b, ch, x_t[(b, ch)])
                synth_pair(pr + 1)
            bs = (2 * pr, 2 * pr + 1)

            # ---------------- depthwise conv ----------------
            for b in bs:
                for ch in range(2):
                    xt = x_t.pop((b, ch))
                    diags = diag_t.pop((b, ch))
                    negW = negW_t.pop((b, ch))
                    Wb = consts["Wb"]
                    ot = outp.tile([128, HW], f32, tag="ot", name="ot")
                    pts = [convps.tile([128, 1024], f32, tag="cps",
                                       name=f"cpt{j}") for j in range(2)]
                    pts.append(convps2.tile([128, 256], f32, tag="cp2",
                                            name="cpt2"))
                    # every tap covers all HW outputs as flat stride-1
                    # matmuls over the padded x; junk in out cols 0 / W-1
                    # (row-edge wrap) is subtracted after evacuation.
                    for idx, ti in enumerate(TAP_ORDER):
                        dy, dx = TAPS[ti]
                        sh = XPAD + W * dy + dx
                        for (e0, clen), pt in zip(CHUNKS, pts):
                            for s0 in range(0, clen, 512):
                                n = min(512, clen - s0)
                                nc.tensor.matmul(
                                    out=pt[:, s0:s0 + n],
                                    lhsT=diags[ti][:],
                                    rhs=xt[:, sh + e0 + s0:sh + e0 + s0 + n],
                                    start=(idx == 0), stop=(idx == NTAP - 1))
                    for (e0, clen), pt in zip(CHUNKS, pts):
                        nc.scalar.activation(
                            out=ot[:, e0:e0 + clen],
                            in_=pt[:, 0:clen],
                            func=AF.Identity,
                            bias=Wb[ch][:, b * 10 + 9:b * 10 + 10],
                            scale=1.0)
                    # fix the two wrapped edge columns (reads x as f32)
                    ov = ot.rearrange("p (h w) -> p h w", w=W)
                    xf = xt.bitcast(f32)[:, XPAD:XPAD + HW].rearrange(
                        "p (h w) -> p h w", w=W)
                    for dy in (-1, 0, 1):
                        # col 0 got x(h+dy-1, W-1) * w[(dy,-1)] where the
                        # source pixel exists
                        ti = (dy + 1) * 3 + 0
                        hs, he = max(0, 1 - dy), H
                        nc.vector.scalar_tensor_tensor(
                            out=ov[:, hs:he, 0:1],
                            in0=xf[:, hs + dy - 1:he + dy - 1, W - 1:W],
                            scalar=negW[:, ti:ti + 1],
                            in1=ov[:, hs:he, 0:1],
                            op0=ALU.mult, op1=ALU.add)
                        # col W-1 got x(h+dy+1, 0) * w[(dy,+1)]
                        ti = (dy + 1) * 3 + 2
                        hs, he = 0, min(H, H - 1 - dy)
                        nc.vector.scalar_tensor_tensor(
                            out=ov[:, hs:he, W - 1:W],
                            in0=xf[:, hs + dy + 1:he + dy + 1, 0:1],
                            scalar=negW[:, ti:ti + 1],
                            in1=ov[:, hs:he, W - 1:W],
                            op0=ALU.mult, op1=ALU.add)
                    nc.scalar.dma_start(
                        out=out_d[b, ch * 128:(ch + 1) * 128].rearrange(
                            "c h w -> c (h w)"),
                        in_=ot[:])


def build_nc(b_loc):
    # Bacc (not raw Bass): its compile() pass splits multi-sem waits into
    # EventSemaphore instructions and moves matmul waits onto LDWEIGHTS —
    # without it walrus rejects Tile output ("Too many sync wait commands").
    nc = bacc.Bacc("TRN2", target_bir_lowering=False, debug=False)
    x_d = nc.dram_tensor("x", [b_loc, C, H, W], f32, kind="ExternalInput").ap()
    w1t_d = nc.dram_tensor("w1t", [C, CR], f32, kind="ExternalInput").ap()
    bns_d = nc.dram_tensor("bns", [CR, 1], f32, kind="ExternalInput").ap()
    bnb_d = nc.dram_tensor("bnb", [CR, 1], f32, kind="ExternalInput").ap()
    w2tb_d = nc.dram_tensor("w2tb", [CR + 1, G * C], f32,
                            kind="ExternalInput").ap()
    dynrep_d = nc.dram_tensor("dynrep", [G, 2, 128, b_loc * 10], f32,
                              kind="ExternalInput").ap()
    ident_d = nc.dram_tensor("ident", [128, 128], f32,
                             kind="ExternalInput").ap()
    out_d = nc.dram_tensor("out", [b_loc, C, H, W], f32,
                           kind="ExternalOutput").ap()
    with tile.TileContext(nc) as tc:
        _body(nc, b_loc, x_d, w1t_d, bns_d, bnb_d, w2tb_d, dynrep_d, ident_d,
              out_d, tc)
    nc.compile()
    return nc


_NC_CACHE = {}


def get_nc(b_loc):
    if b_loc not in _NC_CACHE:
        _NC_CACHE[b_loc] = build_nc(b_loc)
    return _NC_CACHE[b_loc]


def make_host_inputs(b_loc, w1, bn_gamma, bn_beta, bn_mean, bn_var, w2, b2,
                     dyn_weight, dyn_bias):
    w1 = np.asarray(w1, np.float32)
    inv = np.asarray(bn_gamma, np.float32) / np.sqrt(
        np.asarray(bn_var, np.float32) + EPS)
    bns = inv.reshape(CR, 1).astype(np.float32)
    bnb = (np.asarray(bn_beta, np.float32)
           - np.asarray(bn_mean, np.float32) * inv).reshape(CR, 1)
    bnb = bnb.astype(np.float32)
    w1t = np.ascontiguousarray(w1.T / 256.0).astype(np.float32)
    w2tb = np.concatenate(
        [np.asarray(w2, np.float32).T,
         np.asarray(b2, np.float32)[None, :]], axis=0)
    w2tb = np.ascontiguousarray(w2tb).astype(np.float32)
    dw = np.asarray(dyn_weight, np.float32).reshape(G, C, NTAP)
    db = np.asarray(dyn_bias, np.float32)
    dynrep = np.zeros((G, 2, 128, b_loc * 10), np.float32)
    for ch in range(2):
        cs = slice(ch * 128, (ch + 1) * 128)
        for b in range(b_loc):
            dynrep[:, ch, :, b * 10:b * 10 + 9] = dw[:, cs, :]
            dynrep[:, ch, :, b * 10 + 9] = db[:, cs]
    ident = np.eye(128, dtype=np.float32)
    return dict(w1t=w1t, bns=bns, bnb=bnb, w2tb=w2tb, dynrep=dynrep,
                ident=ident)


def kernel(x, w1, bn_gamma, bn_beta, bn_mean, bn_var, w2, b2, dyn_weight,
           dyn_bias):
    x = np.asarray(x, np.float32)
    B = x.shape[0]
    b_loc = B // N_CORES
    nc = get_nc(b_loc)
    params = make_host_inputs(b_loc, w1, bn_gamma, bn_beta, bn_mean, bn_var,
                              w2, b2, dyn_weight, dyn_bias)
    in_maps = []
    for cid in range(N_CORES):
        m = dict(params)
        m["x"] = np.ascontiguousarray(x[cid * b_loc:(cid + 1) * b_loc])
        in_maps.append(m)
    res = run_bass_kernel_spmd(nc, in_maps, list(range(N_CORES)))
    outs = [res.results[i]["out"] for i in range(N_CORES)]
    return np.concatenate(outs, axis=0).astype(np.float32)
